# revision 1
# baseline (speedup 1.0000x reference)
"""Trainium2 Bass kernel for nn_Attention (B=4, SEQ=2048, DIM=1024, H=16).

Sharding: tensor-parallel over heads - 2 heads per core on 8 cores.
Per core: QKV projection (its heads), attention, row-parallel FC partial.
Gather: host sums the 8 partial FC outputs (+ b_fc).

Optimizations over the v1 kernel:
- Host-side key compaction: padding-masked keys contribute exactly zero
  (exp(-1e7)=0 in the reference), so K/V projection, scores, exp and AV run
  only over kept keys per batch (padded to a 128 multiple, baked at build).
- bf16 activations/weights and bf16 partial FC output (host sums in fp64).
- Normalization reciprocals are computed straight from the AV PSUM rowsum
  rows; per-half PE broadcasts avoid cross-partition staging for the mul.
- FC emission is deferred one (qt,qh) step so the xn partition-shift DMA
  latency hides under the next AV block; scores for qt1 are interleaved
  into qt0's AV/FC phase to keep the scalar engine fed.
- DMA queues split across engines: loads on SP, xn-shift on DVE, output
  stores on Pool; FC PSUM->SBUF staging runs on the idle Pool engine.
"""

import sys

sys.path.insert(0, "/opt/trn_rl_repo")

from contextlib import ExitStack

import numpy as np
import ml_dtypes

import concourse.bass as bass
import concourse.tile as tile
from concourse import bacc, mybir
from concourse.bass_utils import run_bass_kernel_spmd

F32 = mybir.dt.float32
F32R = mybir.dt.float32r
BF16 = mybir.dt.bfloat16
BF16_NP = ml_dtypes.bfloat16
FP8 = mybir.dt.float8e4
FP8_NP = ml_dtypes.float8_e4m3
DR = mybir.MatmulPerfMode.DoubleRow

B, SEQ, DIM, H, DH = 4, 2048, 1024, 16, 64
ROWS = B * SEQ  # 8192
SCALE = DH ** -0.5  # 0.125

_CACHE = {}
LAST_RESULTS = None


def _build(kjs):
    """kjs: per-batch number of 128-wide key tiles after compaction."""
    nkj = sum(kjs)
    kps = [k * 128 for k in kjs]
    koff = [sum(kps[:b]) for b in range(B)]  # col offset into xkT
    toff = [sum(kjs[:b]) for b in range(B)]  # tile offset into keep

    nc = bacc.Bacc(
        "TRN2",
        target_bir_lowering=False,
        debug=False,
        enable_asserts=False,
        num_devices=8,
    )
    xTh = nc.dram_tensor("xTh", [DIM, ROWS], FP8, kind="ExternalInput").ap()
    xTl = nc.dram_tensor("xTl", [DIM, ROWS], FP8, kind="ExternalInput").ap()
    xkTh = nc.dram_tensor("xkTh", [DIM, sum(kps)], FP8, kind="ExternalInput").ap()
    xkTl = nc.dram_tensor("xkTl", [DIM, sum(kps)], FP8, kind="ExternalInput").ap()
    wqkvTh = nc.dram_tensor("wqkvTh", [DIM, 384], FP8, kind="ExternalInput").ap()
    wqkvTl = nc.dram_tensor("wqkvTl", [DIM, 384], FP8, kind="ExternalInput").ap()
    wfcT = nc.dram_tensor("wfcT", [128, DIM], F32R, kind="ExternalInput").ap()
    keep = nc.dram_tensor("keep", [128, nkj], F32, kind="ExternalInput").ap()
    eC = nc.dram_tensor("eC", [1, 64], F32R, kind="ExternalInput").ap()
    outp = nc.dram_tensor("outp", [ROWS, DIM], BF16, kind="ExternalOutput").ap()

    EXP = mybir.ActivationFunctionType.Exp

    with tile.TileContext(nc) as tc, ExitStack() as ctx:
        p_const = ctx.enter_context(tc.tile_pool(name="const", bufs=1))
        p_xq = ctx.enter_context(tc.tile_pool(name="xq", bufs=2))
        p_xk = ctx.enter_context(tc.tile_pool(name="xk", bufs=2))
        p_qk = ctx.enter_context(tc.tile_pool(name="qk", bufs=2))
        p_va = ctx.enter_context(tc.tile_pool(name="va", bufs=1))
        p_pt = ctx.enter_context(tc.tile_pool(name="pt", bufs=29))
        p_xn = ctx.enter_context(tc.tile_pool(name="xn", bufs=4))
        p_xnb = ctx.enter_context(tc.tile_pool(name="xnb", bufs=4))
        p_r = ctx.enter_context(tc.tile_pool(name="r", bufs=2))
        p_rps = ctx.enter_context(tc.tile_pool(name="rps", bufs=2))
        p_fco = ctx.enter_context(tc.tile_pool(name="fco", bufs=2))
        p_st = ctx.enter_context(tc.tile_pool(name="st", bufs=2, space="PSUM"))
        p_xa = ctx.enter_context(tc.tile_pool(name="xa", bufs=2, space="PSUM"))
        p_mm = ctx.enter_context(tc.tile_pool(name="mm", bufs=2, space="PSUM"))

        wh_sb = p_const.tile([128, 8 * 384], FP8, tag="wh")
        wl_sb = p_const.tile([128, 8 * 384], FP8, tag="wl")
        for c in range(8):
            nc.sync.dma_start(
                wh_sb[:, c * 384 : (c + 1) * 384],
                wqkvTh[c * 128 : (c + 1) * 128, :],
            )
            nc.sync.dma_start(
                wl_sb[:, c * 384 : (c + 1) * 384],
                wqkvTl[c * 128 : (c + 1) * 128, :],
            )
        w3h = wh_sb[:].rearrange("p (c n) -> p c n", c=8)
        w3l = wl_sb[:].rearrange("p (c n) -> p c n", c=8)

        def load_xq(b):
            """x tiles for Q projection: per qt, [128, 8c x 1024] fp8 hi/lo."""
            t = {}
            for h in range(2):
                cs = slice(b * SEQ + h * 1024, b * SEQ + (h + 1) * 1024)
                for tagv, src in (("xqh", xTh), ("xql", xTl)):
                    xt = p_xq.tile([128, 8 * 1024], FP8, tag=tagv)
                    nc.sync.dma_start(
                        xt[:].rearrange("p (c n) -> p c n", c=8),
                        src[:, cs].rearrange("(c p) n -> p c n", c=8),
                    )
                    t[(h, tagv[-1])] = xt
            return t

        def load_xk(b):
            t = {}
            cs = slice(koff[b], koff[b] + kps[b])
            for tagv, src in (("xkh", xkTh), ("xkl", xkTl)):
                xkt = p_xk.tile([128, 8 * kps[b]], FP8, tag=tagv)
                nc.sync.dma_start(
                    xkt[:].rearrange("p (c n) -> p c n", c=8),
                    src[:, cs].rearrange("(c p) n -> p c n", c=8),
                )
                t[tagv[-1]] = xkt
            return t

        xk_t = load_xk(0)
        xq_t = load_xq(0)
        wfc_sb = p_const.tile([128, DIM], F32R, tag="wfc")
        nc.sync.dma_start(wfc_sb[:], wfcT[:])
        keep_sb = p_const.tile([128, nkj], F32, tag="keep")
        nc.sync.dma_start(keep_sb[:], keep[:])
        e_sb = p_const.tile([128, 64], F32R, tag="e")
        nc.sync.dma_start(e_sb[0:1, :], eC[:])
        # wfc rows 64..127 remapped to partitions 0..63 (for shift-free FC
        # of the final tiles)
        wfcB_sb = p_const.tile([64, DIM], F32R, tag="wfcB")
        nc.sync.dma_start(wfcB_sb[:], wfcT[64:128, :])

        fc_queue = []

        def emit_fc(ss=None):
            if not fc_queue:
                return
            xn, xnb, b, qt, qh, split = fc_queue.pop(0)
            fo = p_fco.tile([128, 4 * DIM], BF16, tag="fo")
            for qq in range(4):
                if ss is not None and qq % 2 == 1:
                    ss.pump(1)
                for ot in range(2):
                    fp_ = p_mm.tile([128, 512], F32, tag="mm")
                    if split:
                        # shift-free FC: two K=64 accumulating matmuls
                        nc.tensor.matmul(
                            fp_[:],
                            xn[0:64, qq * 128 : (qq + 1) * 128],
                            wfc_sb[0:64, ot * 512 : (ot + 1) * 512],
                            start=True,
                            stop=False,
                        )
                        nc.tensor.matmul(
                            fp_[:],
                            xnb[:, qq * 128 : (qq + 1) * 128],
                            wfcB_sb[:, ot * 512 : (ot + 1) * 512],
                            start=False,
                            stop=True,
                        )
                    else:
                        nc.tensor.matmul(
                            fp_[:],
                            xn[:, qq * 128 : (qq + 1) * 128],
                            wfc_sb[:, ot * 512 : (ot + 1) * 512],
                            start=True,
                            stop=True,
                        )
                    if ot == 0:
                        nc.vector.tensor_copy(
                            fo[:, qq * DIM : qq * DIM + 512], fp_[:]
                        )
                    else:
                        nc.scalar.copy(
                            fo[:, qq * DIM + 512 : qq * DIM + 1024], fp_[:]
                        )
                row0 = b * SEQ + qt * 1024 + qh * 512 + qq * 128
                nc.sync.dma_start(
                    outp[row0 : row0 + 128, :],
                    fo[:, qq * DIM : (qq + 1) * DIM],
                )


        class ScoreStream:
            """Pending score tiles for one batch, woven into PE-dense spots."""

            def __init__(self, kj_b, qts, kT2, pts):
                self.jobs = [
                    (qt, a, kj)
                    for qt in range(2)
                    for a in range(2)
                    for kj in range(kj_b)
                ]
                self.qts = qts  # (qTa, qTb)
                self.kT2 = kT2
                self.pts = pts

            def _emit(self, qt, a, kj):
                qS = self.qts[qt]
                st = p_st.tile([128, 1024], F32, tag="st")
                for hh in range(2):
                    nc.tensor.matmul(
                        st[:, hh * 512 : (hh + 1) * 512],
                        self.kT2[a * 64 : (a + 1) * 64, kj * 128 : (kj + 1) * 128],
                        qS[a * 64 : (a + 1) * 64, hh * 512 : (hh + 1) * 512],
                        start=True,
                        stop=True,
                        skip_group_check=True,
                    )
                pt = p_pt.tile([128, 1024], BF16, tag="pt")
                nc.scalar.activation(pt[:], st[:], EXP, scale=SCALE / 256.0)
                self.pts[(qt, a, kj)] = pt

            def pump(self, n=1):
                while n > 0 and self.jobs:
                    self._emit(*self.jobs.pop(0))
                    n -= 1

            def flush_until(self, qt, a):
                while self.jobs and self.jobs[0][:2] <= (qt, a):
                    self._emit(*self.jobs.pop(0))

        for b in range(B):
            kj_b = kjs[b]
            kp_b = kps[b]

            # ---- K projection over compacted keys (scores depend on it)
            kT2 = p_qk.tile([128, kp_b], BF16, tag="k")
            xk3h = xk_t["h"][:].rearrange("p (c n) -> p c n", c=8)
            xk3l = xk_t["l"][:].rearrange("p (c n) -> p c n", c=8)
            n0 = 0
            while n0 < kp_b:
                n1 = min(n0 + 512, kp_b)
                ps = p_mm.tile([128, 512], F32, tag="mm")
                first = True
                for wv, xv in ((w3h, xk3h), (w3h, xk3l), (w3l, xk3h)):
                    for cp in range(0, 8, 2):
                        nc.tensor.matmul(
                            ps[:, : n1 - n0],
                            wv[:, cp : cp + 2, 128:256],
                            xv[:, cp : cp + 2, n0:n1],
                            start=first,
                            stop=(wv is w3l and cp == 6),
                            perf_mode=DR,
                        )
                        first = False
                nc.vector.tensor_copy(kT2[:, n0:n1], ps[:, : n1 - n0])
                n0 = n1

            # ---- Q projection into per-qt tiles qTa/qTb [128ch, 1024] bf16
            pts = {}
            qts = []
            ss = None
            for qt in range(2):
                qS = p_qk.tile([128, 1024], BF16, tag=f"q{qt}")
                xq3h = xq_t[(qt, "h")][:].rearrange("p (c n) -> p c n", c=8)
                xq3l = xq_t[(qt, "l")][:].rearrange("p (c n) -> p c n", c=8)
                for n in range(2):
                    ps = p_mm.tile([128, 512], F32, tag="mm")
                    first = True
                    for wv, xv in ((w3h, xq3h), (w3h, xq3l), (w3l, xq3h)):
                        for cp in range(0, 8, 2):
                            nc.tensor.matmul(
                                ps[:],
                                wv[:, cp : cp + 2, 0:128],
                                xv[:, cp : cp + 2, n * 512 : (n + 1) * 512],
                                start=first,
                                stop=(wv is w3l and cp == 6),
                                perf_mode=DR,
                            )
                            first = False
                    nc.vector.tensor_copy(qS[:, n * 512 : (n + 1) * 512], ps[:])
                    if ss is not None:
                        ss.pump(1)
                qts.append(qS)
                if qt == 0:
                    # qt0 scores can start as soon as qTa + kT2 exist;
                    # drain a deferred FC here to give the PE work while
                    # the scalar engine chews the first exps
                    ss = ScoreStream(kj_b, qts, kT2, pts)
                    if len(fc_queue) > 2:
                        emit_fc(ss)

            if len(fc_queue) > 2:
                emit_fc(ss)
            if len(fc_queue) > 2:
                emit_fc(ss)

            # prefetch next batch inputs (bufs rotate as QKV consumes)
            if b + 1 < B:
                nxk = load_xk(b + 1)
                nxq = load_xq(b + 1)

            # ---- V projected directly into [keys, d] layout, keep-scaled
            # augmented V  [128k, kj*130] bf16; weave scores between groups
            va = p_va.tile([128, kj_b * 130], BF16, tag="va")
            for kj in range(kj_b):
                pv = p_mm.tile([128, 128], F32, tag="mm")
                first = True
                for xv, wv in ((xk3h, w3h), (xk3l, w3h), (xk3h, w3l)):
                    for cp in range(0, 8, 2):
                        nc.tensor.matmul(
                            pv[:],
                            xv[:, cp : cp + 2, kj * 128 : (kj + 1) * 128],
                            wv[:, cp : cp + 2, 256:384],
                            start=first,
                            stop=(wv is w3l and cp == 6),
                            perf_mode=DR,
                        )
                        first = False
                kap = keep_sb[:, toff[b] + kj : toff[b] + kj + 1]
                o = kj * 130
                nc.vector.tensor_scalar_mul(va[:, o : o + 64], pv[:, 0:64], kap)
                nc.vector.tensor_copy(va[:, o + 64 : o + 65], kap)
                nc.vector.tensor_scalar_mul(va[:, o + 65 : o + 129], pv[:, 64:128], kap)
                nc.vector.tensor_copy(va[:, o + 129 : o + 130], kap)
                if kj % 2 == 1:
                    ss.pump(1)

            # ---- attention (a-major: both qh of head a before head a+1, so
            # the score weave has twice the slot capacity per flush point)
            for qt in range(2):
                split = (b == B - 1) and (qt == 1)
                xns = [
                    p_xn.tile([128, 512], F32R, tag="xn", name=f"xn{b}{qt}{i}")
                    for i in range(2)
                ]
                xnbs = [
                    p_xnb.tile([64, 512], F32R, tag="xnb", name=f"xnb{b}{qt}{i}")
                    for i in range(2)
                ]

                for a in range(2):
                    for qh in range(2):
                        ss.flush_until(qt, a)
                        xa = p_xa.tile([65, 512], F32, tag="xa")
                        for kj in range(kj_b):
                            o = kj * 130 + a * 65
                            nc.tensor.matmul(
                                xa[:],
                                va[:, o : o + 65],
                                pts[(qt, a, kj)][:, qh * 512 : (qh + 1) * 512],
                                start=(kj == 0),
                                stop=(kj == kj_b - 1),
                                skip_group_check=True,
                            )
                            if kj % 4 == 3:
                                ss.pump(1)
                        # normalize this head's half right away:
                        # reciprocal of the rowsum row (PSUM row 64),
                        # PE-broadcast onto partitions 0..63, stage the
                        # broadcast to SBUF (single-PSUM-input rule), multiply.
                        rsum = p_r.tile([128, 512], F32, tag="rsum")
                        nc.vector.tensor_copy(rsum[64:65, :], xa[64:65, :])
                        # rowsum row to partition 0 (DMA partition shift)
                        rr = p_r.tile([128, 512], F32, tag="rr")
                        nc.sync.dma_start(rr[0:1, :], rsum[64:65, :])
                        rri = p_r.tile([128, 512], F32, tag="rri")
                        nc.vector.reciprocal_approx_fast(rri[0:1, :], rr[0:1, :])
                        rrc = p_r.tile([128, 512], F32R, tag="rrc")
                        nc.vector.tensor_copy(rrc[0:1, :], rri[0:1, :])
                        Rp = p_mm.tile([128, 512], F32, tag="mm")
                        nc.tensor.matmul(
                            Rp[0:64, :], e_sb[0:1, :], rrc[0:1, :],
                            start=True, stop=True,
                        )
                        Rps = p_rps.tile([64, 512], F32, tag="rps")
                        nc.vector.tensor_copy(Rps[:], Rp[0:64, :])
                        if a == 0:
                            nc.vector.tensor_mul(
                                xns[qh][0:64, :], xa[0:64, :], Rps[:]
                            )
                        else:
                            nc.vector.tensor_mul(
                                xnbs[qh][:], xa[0:64, :], Rps[:]
                            )
                            if not split:
                                nc.sync.dma_start(
                                    xns[qh][64:128, :].bitcast(F32),
                                    xnbs[qh][:].bitcast(F32),
                                )
                            fc_queue.append(
                                (xns[qh], xnbs[qh], b, qt, qh, split)
                            )
                            if len(fc_queue) > 2:
                                emit_fc(ss)

            if b + 1 < B:
                xq_t, xk_t = nxq, nxk

        while fc_queue:
            emit_fc()

    nc.compile()
    return nc


def _prep_inputs(inputs, W_qkv, W_fc, padding_mask, kjs):
    kps = [k * 128 for k in kjs]
    x2 = np.asarray(inputs, np.float32).reshape(ROWS, DIM)

    def hilo(a):
        hi = a.astype(FP8_NP)
        lo = (a - hi.astype(np.float32)).astype(FP8_NP)
        return hi, lo

    xT_f = np.ascontiguousarray(x2.T)
    xTh_np, xTl_np = hilo(xT_f)
    Wq = np.asarray(W_qkv, np.float32)
    Wf = np.asarray(W_fc, np.float32)
    mask = np.asarray(padding_mask)

    xk_rows = []
    keep_cols = []
    for b in range(B):
        idx = np.nonzero(mask[b] == 0)[0]
        kp = kps[b]
        rows = np.zeros((kp, DIM), np.float32)
        rows[: len(idx)] = x2[b * SEQ + idx]
        xk_rows.append(rows)
        kv = np.zeros(kp, np.float32)
        kv[: len(idx)] = 1.0
        keep_cols.append(kv.reshape(kjs[b], 128).T)  # [128, kj_b]
    xkT_f = np.ascontiguousarray(np.concatenate(xk_rows, axis=0).T)
    xkTh_np, xkTl_np = hilo(xkT_f)
    keep_np = np.ascontiguousarray(np.concatenate(keep_cols, axis=1))

    eCv = np.ones((1, 64), np.float32)
    in_maps = []
    for i in range(8):
        h0 = 2 * i
        rows = np.concatenate(
            [
                Wq[h0 * 64 : (h0 + 2) * 64],
                Wq[DIM + h0 * 64 : DIM + (h0 + 2) * 64],
                Wq[2 * DIM + h0 * 64 : 2 * DIM + (h0 + 2) * 64],
            ],
            axis=0,
        )  # [384, 1024]
        wT_f = np.ascontiguousarray(rows.T) * 16.0
        wh_np, wl_np = hilo(wT_f)
        in_maps.append(
            {
                "xTh": xTh_np,
                "xTl": xTl_np,
                "xkTh": xkTh_np,
                "xkTl": xkTl_np,
                "wqkvTh": wh_np,
                "wqkvTl": wl_np,
                "wfcT": np.ascontiguousarray(Wf[:, i * 128 : (i + 1) * 128].T) / 16.0,
                "keep": keep_np,
                "eC": eCv,
            }
        )
    return in_maps


def kernel(inputs, W_qkv, W_fc, b_fc, padding_mask, trace=False, trace_kwargs=None):
    global LAST_RESULTS
    mask = np.asarray(padding_mask)
    kjs = tuple(
        max(1, int(np.ceil((mask[b] == 0).sum() / 128))) for b in range(B)
    )
    if kjs not in _CACHE:
        _CACHE[kjs] = _build(kjs)
    nc = _CACHE[kjs]
    _CACHE["nc"] = nc  # last-used, for external profiling
    in_maps = _prep_inputs(inputs, W_qkv, W_fc, padding_mask, kjs)
    kw = {}
    if trace:
        kw["trace"] = True
        if trace_kwargs:
            kw.update(trace_kwargs)
    res = run_bass_kernel_spmd(nc, in_maps, core_ids=list(range(8)), **kw)
    LAST_RESULTS = res
    acc = np.zeros((ROWS, DIM), np.float64)
    for r in res.results:
        acc += r["outp"].astype(np.float64)
    acc += np.asarray(b_fc, np.float64)[None, :]
    return acc.astype(np.float32).reshape(B, SEQ, DIM)



# revision 18
# speedup vs baseline: 1.4000x; 1.4000x over previous
"""Trainium2 Bass kernel for nn_Attention (B=4, SEQ=2048, DIM=1024, H=16).

Sharding v2: (batch x head-half) - core i handles batch i%4, heads
(i//4)*8 .. +8 (four head-pairs). Host sums the 2 partial FC outputs
per batch (+ b_fc). Versus the v1 all-batches-per-core tensor-parallel
split this keeps PE work identical but cuts per-core DMA ~4x (each
core loads only its batch) and removes the HWDGE/SP-queue contention.

Kernel structure per core:
- Host-side key compaction (padding-masked keys contribute exactly 0).
- QKV projections in fp8 hi/lo x DoubleRow (3 passes, 0.5 cyc/col).
- Scores (K=64) and AV (K=128) in bf16; exp on ACT engine.
- Softmax normalization: the AV PSUM rowsum row (augmented-V trick) is
  partition-broadcast on the idle GPSIMD engine directly from PSUM,
  reciprocal'd on DVE, and multiplied into xn - no PE broadcast matmul,
  no staging copies, no partition-shift DMA for the rowsum.
- FC: bf16, 4 accumulating K=128 chunk matmuls per output tile, chunk
  order chosen so the head-b partition-shift DMAs hide under the spine.
- Score exps are paced into AV/Qproj/FC gaps (~1.2us apart) so the ACT
  engine never back-pressures the in-order PE queue via the 2-deep
  score-PSUM pool.
"""

import sys

sys.path.insert(0, "/opt/trn_rl_repo")

from contextlib import ExitStack

import numpy as np
import ml_dtypes

import concourse.bass as bass
import concourse.tile as tile
from concourse import bacc, mybir
from concourse.bass_utils import run_bass_kernel_spmd

F32 = mybir.dt.float32
BF16 = mybir.dt.bfloat16
BF16_NP = ml_dtypes.bfloat16
FP8 = mybir.dt.float8e4
FP8_NP = ml_dtypes.float8_e4m3
DR = mybir.MatmulPerfMode.DoubleRow

B, SEQ, DIM, H, DH = 4, 2048, 1024, 16, 64
SCALE = DH ** -0.5  # 0.125
NHP = 4  # head-pairs per core (8 heads)

_CACHE = {}
LAST_RESULTS = None


def _build(kj):
    """kj: number of 128-wide key tiles after compaction (uniform, padded)."""
    KP = kj * 128

    nc = bacc.Bacc(
        "TRN2",
        target_bir_lowering=False,
        debug=False,
        enable_asserts=False,
        num_devices=8,
    )
    xTh = nc.dram_tensor("xTh", [DIM, SEQ], FP8, kind="ExternalInput").ap()
    xTl = nc.dram_tensor("xTl", [DIM, SEQ], FP8, kind="ExternalInput").ap()
    xkTh = nc.dram_tensor("xkTh", [DIM, KP], FP8, kind="ExternalInput").ap()
    xkTl = nc.dram_tensor("xkTl", [DIM, KP], FP8, kind="ExternalInput").ap()
    wqkvTh = nc.dram_tensor("wqkvTh", [DIM, 384 * NHP], FP8, kind="ExternalInput").ap()
    wqkvTl = nc.dram_tensor("wqkvTl", [DIM, 384 * NHP], FP8, kind="ExternalInput").ap()
    wfcT = nc.dram_tensor("wfcT", [128, NHP * DIM], BF16, kind="ExternalInput").ap()
    keep = nc.dram_tensor("keep", [128, kj], F32, kind="ExternalInput").ap()
    outp = nc.dram_tensor("outp", [SEQ, DIM], BF16, kind="ExternalOutput").ap()

    EXP = mybir.ActivationFunctionType.Exp

    with tile.TileContext(nc) as tc, ExitStack() as ctx:
        p_const = ctx.enter_context(tc.tile_pool(name="const", bufs=1))
        p_xq = ctx.enter_context(tc.tile_pool(name="xq", bufs=2))
        p_xk = ctx.enter_context(tc.tile_pool(name="xk", bufs=1))
        p_k = ctx.enter_context(tc.tile_pool(name="k", bufs=1))
        p_va = ctx.enter_context(tc.tile_pool(name="va", bufs=1))
        p_q = ctx.enter_context(tc.tile_pool(name="q", bufs=3))
        p_pt = ctx.enter_context(tc.tile_pool(name="pt", bufs=26))
        p_xn = ctx.enter_context(tc.tile_pool(name="xn", bufs=8))
        p_xnb = ctx.enter_context(tc.tile_pool(name="xnb", bufs=4))
        p_rb = ctx.enter_context(tc.tile_pool(name="rb", bufs=3))
        p_fo = ctx.enter_context(tc.tile_pool(name="fo", bufs=3))
        p_st = ctx.enter_context(tc.tile_pool(name="st", bufs=2, space="PSUM"))
        p_xa = ctx.enter_context(tc.tile_pool(name="xa", bufs=2, space="PSUM"))
        p_mm = ctx.enter_context(tc.tile_pool(name="mm", bufs=2, space="PSUM"))

        # ---- constant + input loads (SP queue) ----
        wh_sb = p_const.tile([128, 8 * 384 * NHP], FP8, tag="wh")
        wl_sb = p_const.tile([128, 8 * 384 * NHP], FP8, tag="wl")
        nc.sync.dma_start(
            wh_sb[:].rearrange("p (c n) -> p c n", c=8),
            wqkvTh[:].rearrange("(c p) n -> p c n", c=8),
        )
        xkh_sb = p_xk.tile([128, 8 * KP], FP8, tag="xkh")
        xkl_sb = p_xk.tile([128, 8 * KP], FP8, tag="xkl")
        nc.sync.dma_start(
            xkh_sb[:].rearrange("p (c n) -> p c n", c=8),
            xkTh[:].rearrange("(c p) n -> p c n", c=8),
        )
        nc.sync.dma_start(
            xkl_sb[:].rearrange("p (c n) -> p c n", c=8),
            xkTl[:].rearrange("(c p) n -> p c n", c=8),
        )
        nc.sync.dma_start(
            wl_sb[:].rearrange("p (c n) -> p c n", c=8),
            wqkvTl[:].rearrange("(c p) n -> p c n", c=8),
        )
        w3h = wh_sb[:].rearrange("p (c n) -> p c n", c=8)
        w3l = wl_sb[:].rearrange("p (c n) -> p c n", c=8)
        xk3h = xkh_sb[:].rearrange("p (c n) -> p c n", c=8)
        xk3l = xkl_sb[:].rearrange("p (c n) -> p c n", c=8)

        def load_xq(qt):
            t = {}
            cs = slice(qt * 1024, (qt + 1) * 1024)
            for tagv, src in (("xqh", xTh), ("xql", xTl)):
                xt = p_xq.tile([128, 8 * 1024], FP8, tag=tagv)
                nc.sync.dma_start(
                    xt[:].rearrange("p (c n) -> p c n", c=8),
                    src[:, cs].rearrange("(c p) n -> p c n", c=8),
                )
                t[tagv[-1]] = xt
            return t

        xq_t = {0: load_xq(0)}
        keep_sb = p_const.tile([128, kj], F32, tag="keep")
        nc.sync.dma_start(keep_sb[:], keep[:])
        xq_t[1] = load_xq(1)
        wfc_sb = p_const.tile([128, NHP * DIM], BF16, tag="wfc")
        nc.sync.dma_start(wfc_sb[:], wfcT[:])

        # ---- K projection: kT2[hp] = [128 (2 heads x 64 dh), KP] bf16 ----
        def kproj(hp):
            kT2 = p_k.tile([128, KP], BF16, tag=f"k{hp}")
            wo = hp * 384 + 128
            n0 = 0
            while n0 < KP:
                n1 = min(n0 + 512, KP)
                ps = p_mm.tile([128, 512], F32, tag="mm")
                first = True
                for wv, xv in ((w3h, xk3h), (w3h, xk3l), (w3l, xk3h)):
                    for cp in range(0, 8, 2):
                        nc.tensor.matmul(
                            ps[:, : n1 - n0],
                            wv[:, cp : cp + 2, wo : wo + 128],
                            xv[:, cp : cp + 2, n0:n1],
                            start=first,
                            stop=(wv is w3l and cp == 6),
                            perf_mode=DR,
                        )
                        first = False
                nc.scalar.copy(kT2[:, n0:n1], ps[:, : n1 - n0])
                n0 = n1
            return kT2

        # ---- V projection into keep-scaled augmented layout ----
        # va[hp] columns per key tile t: [v_a(64)*keep, keep, v_b(64)*keep, keep]
        def vproj(hp, pump):
            va = p_va.tile([128, kj * 130], BF16, tag=f"va{hp}")
            wo = hp * 384 + 256
            for t in range(kj):
                pv = p_mm.tile([128, 128], F32, tag="mm")
                first = True
                for xv, wv in ((xk3h, w3h), (xk3l, w3h), (xk3h, w3l)):
                    for cp in range(0, 8, 2):
                        nc.tensor.matmul(
                            pv[:],
                            xv[:, cp : cp + 2, t * 128 : (t + 1) * 128],
                            wv[:, cp : cp + 2, wo : wo + 128],
                            start=first,
                            stop=(wv is w3l and cp == 6),
                            perf_mode=DR,
                        )
                        first = False
                kap = keep_sb[:, t : t + 1]
                o = t * 130
                nc.vector.tensor_scalar_mul(va[:, o : o + 64], pv[:, 0:64], kap)
                nc.vector.tensor_copy(va[:, o + 64 : o + 65], kap)
                nc.vector.tensor_scalar_mul(va[:, o + 65 : o + 129], pv[:, 64:128], kap)
                nc.vector.tensor_copy(va[:, o + 129 : o + 130], kap)
                if t % 2 == 1:
                    pump(1)
            return va

        # ---- Q projection: qS = [128 (2 heads x 64 dh), 1024 queries] ----
        def qproj(qt, hp, pump):
            qS = p_q.tile([128, 1024], BF16, tag="q")
            wo = hp * 384
            xq3h = xq_t[qt]["h"][:].rearrange("p (c n) -> p c n", c=8)
            xq3l = xq_t[qt]["l"][:].rearrange("p (c n) -> p c n", c=8)
            for n in range(2):
                ps = p_mm.tile([128, 512], F32, tag="mm")
                first = True
                for wv, xv in ((w3h, xq3h), (w3h, xq3l), (w3l, xq3h)):
                    for cp in range(0, 8, 2):
                        nc.tensor.matmul(
                            ps[:],
                            wv[:, cp : cp + 2, wo : wo + 128],
                            xv[:, cp : cp + 2, n * 512 : (n + 1) * 512],
                            start=first,
                            stop=(wv is w3l and cp == 6),
                            perf_mode=DR,
                        )
                        first = False
                        if wv is w3h and xv is xq3l and cp == 6:
                            pump(1)
                nc.vector.tensor_copy(qS[:, n * 512 : (n + 1) * 512], ps[:])
                pump(1)
            return qS

        pts = {}
        xns = {}
        kT2s = []
        vas = []
        fc_queue = []

        class SS:
            """Pending score+exp tiles for one (qt, hp), paced into PE gaps."""

            def __init__(self, qt, hp, kT2, qS):
                self.qt, self.hp, self.kT2, self.qS = qt, hp, kT2, qS
                self.jobs = [(a, t) for a in range(2) for t in range(kj)]

            def _emit(self, a, t):
                st = p_st.tile([128, 1024], F32, tag="st")
                for hh in range(2):
                    nc.tensor.matmul(
                        st[:, hh * 512 : (hh + 1) * 512],
                        self.kT2[a * 64 : (a + 1) * 64, t * 128 : (t + 1) * 128],
                        self.qS[a * 64 : (a + 1) * 64, hh * 512 : (hh + 1) * 512],
                        start=True,
                        stop=True,
                        skip_group_check=True,
                    )
                pt = p_pt.tile([128, 1024], BF16, tag="pt")
                nc.scalar.activation(pt[:], st[:], EXP, scale=SCALE / 256.0)
                pts[(self.qt, self.hp, a, t)] = pt

            def pump(self, n=1):
                while n > 0 and self.jobs:
                    self._emit(*self.jobs.pop(0))
                    n -= 1

            def flush_all(self):
                while self.jobs:
                    self._emit(*self.jobs.pop(0))

        def emit_fc(ss=None):
            if not fc_queue:
                return
            qt, qq = fc_queue.pop(0)
            last = qt == 1 and not fc_queue
            order = (0, 1, 2, 3) if qt == 0 else (3, 0, 1, 2)
            fo = p_fo.tile([128, DIM], BF16, tag="fo")
            r0 = qt * 1024 + qq * 128
            for ot in range(2):
                fp = p_mm.tile([128, 512], F32, tag="mm")
                for j, hp in enumerate(order):
                    nc.tensor.matmul(
                        fp[:],
                        xns[(qt, hp)][:, qq * 128 : (qq + 1) * 128],
                        wfc_sb[:, hp * DIM + ot * 512 : hp * DIM + ot * 512 + 512],
                        start=(j == 0),
                        stop=(j == 3),
                    )
                nc.vector.tensor_copy(fo[:, ot * 512 : (ot + 1) * 512], fp[:])
                if last:
                    # final tile: store each half as soon as it is staged
                    nc.gpsimd.dma_start(
                        outp[r0 : r0 + 128, ot * 512 : (ot + 1) * 512],
                        fo[:, ot * 512 : (ot + 1) * 512],
                    )
                if ss is not None:
                    ss.pump(1)
            if not last:
                nc.gpsimd.dma_start(outp[r0 : r0 + 128, :], fo[:])

        def av_block(qt, hp, ss_next):
            xn = p_xn.tile([128, 1024], BF16, tag="xn", name=f"xn{qt}{hp}")
            xns[(qt, hp)] = xn
            va = vas[hp]
            for a in range(2):
                for qh in range(2):
                    xa = p_xa.tile([65, 512], F32, tag="xa")
                    for t in range(kj):
                        o = t * 130 + a * 65
                        nc.tensor.matmul(
                            xa[:],
                            va[:, o : o + 65],
                            pts[(qt, hp, a, t)][:, qh * 512 : (qh + 1) * 512],
                            start=(t == 0),
                            stop=(t == kj - 1),
                            skip_group_check=True,
                        )
                        if t % 4 == 3 and ss_next is not None and len(ss_next.jobs) > 10:
                            ss_next.pump(1)
                    # normalize: reciprocal of the PSUM rowsum row on DVE
                    # (DVE may read PSUM), partition-broadcast on GPSIMD,
                    # multiply into xn.
                    rr = p_rb.tile([1, 512], F32, tag="rr")
                    nc.vector.reciprocal_approx_fast(rr[0:1, :], xa[64:65, :])
                    ri = p_rb.tile([64, 512], F32, tag="ri")
                    nc.gpsimd.partition_broadcast(ri[:], rr[0:1, :])
                    if a == 0:
                        nc.vector.tensor_mul(
                            xn[0:64, qh * 512 : (qh + 1) * 512], xa[0:64, :], ri[:]
                        )
                    else:
                        xnb = p_xnb.tile([64, 512], BF16, tag="xnb")
                        nc.vector.tensor_mul(xnb[:], xa[0:64, :], ri[:])
                        # shift DMA on SP (stores go via GPSIMD SWDGE so they
                        # cannot delay these latency-critical shifts).
                        nc.sync.dma_start(
                            xn[64:128, qh * 512 : (qh + 1) * 512], xnb[:]
                        )
                        emit_fc(ss_next)
                        emit_fc(ss_next)
                    if ss_next is not None:
                        ss_next.pump(3)

        # hp0+hp1 K projections pass-major: the hh-pass matmuls for all
        # chunks of both head-pairs run as soon as (wh, xkh) land, the
        # hl-pass when xkl lands, the lh-pass when wl lands - instead of
        # the whole pipeline stalling on the last of the four loads.
        def kproj01():
            res, ps2, pc2 = [], {}, {}
            nch = [
                (i * 512, min((i + 1) * 512, KP))
                for i in range((KP + 511) // 512)
            ]
            for hp in (0, 1):
                res.append(
                    p_k.tile([128, KP], BF16, tag=f"k{hp}", name=f"kT2_{hp}")
                )
                ps2[hp] = p_st.tile([128, 1024], F32, tag="st", name=f"kps{hp}")
                if KP > 1024:
                    pc2[hp] = p_mm.tile([128, 512], F32, tag="mm", name=f"kpc{hp}")
            for pi, (wv, xv) in enumerate(
                ((w3h, xk3h), (w3h, xk3l), (w3l, xk3h))
            ):
                for hp in (0, 1):
                    wo = hp * 384 + 128
                    for ci, (n0, n1) in enumerate(nch):
                        out = (
                            ps2[hp][:, n0:n1]
                            if ci < 2
                            else pc2[hp][:, : n1 - n0]
                        )
                        for cp in range(0, 8, 2):
                            nc.tensor.matmul(
                                out,
                                wv[:, cp : cp + 2, wo : wo + 128],
                                xv[:, cp : cp + 2, n0:n1],
                                start=(pi == 0 and cp == 0),
                                stop=(pi == 2 and cp == 6),
                                perf_mode=DR,
                                skip_group_check=True,
                            )
            for hp in (0, 1):
                e = min(KP, 1024)
                # ACT is idle during startup - keep DVE free for qS copies
                nc.scalar.copy(res[hp][:, 0:e], ps2[hp][:, 0:e])
                if KP > 1024:
                    nc.scalar.copy(res[hp][:, 1024:KP], pc2[hp][:, : KP - 1024])
            return res

        # ================= spine =================
        if KP <= 1536:
            kT2s.extend(kproj01())
        else:
            kT2s.append(kproj(0))
            kT2s.append(kproj(1))
        kT2s.append(kproj(2))
        kT2s.append(kproj(3))

        qS0 = qproj(0, 0, lambda n: None)
        S = {(0, 0): SS(0, 0, kT2s[0], qS0)}
        qSs = {(0, 0): qS0}

        for hp in range(NHP):
            vas.append(vproj(hp, S[(0, 0)].pump))
        S[(0, 0)].flush_all()

        prev = (0, 0)
        seq = [(0, 1), (0, 2), (0, 3), (1, 3), (1, 0), (1, 1), (1, 2)]
        for qt, hp in seq:
            qS = qproj(qt, hp, S[prev].pump)
            qSs[(qt, hp)] = qS
            cur = SS(qt, hp, kT2s[hp], qS)
            S[(qt, hp)] = cur
            S[prev].flush_all()
            av_block(prev[0], prev[1], cur)
            if prev == (0, 3):
                fc_queue.extend((0, qq) for qq in range(8))
            while len(fc_queue) > 4:
                emit_fc(cur)
            prev = (qt, hp)

        S[prev].flush_all()
        fc_queue.extend((1, qq) for qq in range(8))
        av_block(prev[0], prev[1], None)
        while fc_queue:
            emit_fc()

    nc.compile()
    return nc


def _hilo(a):
    hi = a.astype(FP8_NP)
    lo = (a - hi.astype(np.float32)).astype(FP8_NP)
    return hi, lo


def _prep_inputs(inputs, W_qkv, W_fc, padding_mask, kj):
    KP = kj * 128
    x = np.asarray(inputs, np.float32)
    Wq = np.asarray(W_qkv, np.float32)
    Wf = np.asarray(W_fc, np.float32)
    mask = np.asarray(padding_mask)

    xT, xkT, keepc = {}, {}, {}
    for b in range(B):
        xb = x[b]
        xT[b] = _hilo(np.ascontiguousarray(xb.T))
        idx = np.nonzero(mask[b] == 0)[0]
        rows = np.zeros((KP, DIM), np.float32)
        rows[: len(idx)] = xb[idx]
        xkT[b] = _hilo(np.ascontiguousarray(rows.T))
        kv = np.zeros(KP, np.float32)
        kv[: len(idx)] = 1.0
        keepc[b] = np.ascontiguousarray(kv.reshape(kj, 128).T)

    in_maps = []
    for i in range(8):
        b, hs = i % 4, i // 4
        qrs = Wq[hs * 512 : (hs + 1) * 512]
        krs = Wq[DIM + hs * 512 : DIM + (hs + 1) * 512]
        vrs = Wq[2 * DIM + hs * 512 : 2 * DIM + (hs + 1) * 512]
        blocks = []
        for hp in range(NHP):
            blocks += [
                qrs[hp * 128 : (hp + 1) * 128],
                krs[hp * 128 : (hp + 1) * 128],
                vrs[hp * 128 : (hp + 1) * 128],
            ]
        wT = np.ascontiguousarray(np.concatenate(blocks, axis=0).T) * 16.0
        wh, wl = _hilo(wT)
        wfcT = np.concatenate(
            [
                np.ascontiguousarray(
                    Wf[:, hs * 512 + hp * 128 : hs * 512 + (hp + 1) * 128].T
                )
                for hp in range(NHP)
            ],
            axis=1,
        ) / 16.0
        in_maps.append(
            {
                "xTh": xT[b][0],
                "xTl": xT[b][1],
                "xkTh": xkT[b][0],
                "xkTl": xkT[b][1],
                "wqkvTh": wh,
                "wqkvTl": wl,
                "wfcT": wfcT.astype(BF16_NP),
                "keep": keepc[b],
            }
        )
    return in_maps


def kernel(inputs, W_qkv, W_fc, b_fc, padding_mask, trace=False, trace_kwargs=None):
    global LAST_RESULTS
    mask = np.asarray(padding_mask)
    kj = max(
        1, max(int(np.ceil((mask[b] == 0).sum() / 128)) for b in range(B))
    )
    if kj not in _CACHE:
        _CACHE[kj] = _build(kj)
    nc = _CACHE[kj]
    _CACHE["nc"] = nc  # last-used, for external profiling
    in_maps = _prep_inputs(inputs, W_qkv, W_fc, padding_mask, kj)
    kw = {}
    if trace:
        kw["trace"] = True
        if trace_kwargs:
            kw.update(trace_kwargs)
    res = run_bass_kernel_spmd(nc, in_maps, core_ids=list(range(8)), **kw)
    LAST_RESULTS = res
    out = np.empty((B, SEQ, DIM), np.float32)
    bfc = np.asarray(b_fc, np.float32)[None, :]
    for b in range(B):
        out[b] = (
            res.results[b]["outp"].astype(np.float32)
            + res.results[b + 4]["outp"].astype(np.float32)
            + bfc
        )
    return out
